# revision 43
# baseline (speedup 1.0000x reference)
"""GAT encoder (10-layer, JK-concat) Trainium2 Bass kernel — 8-core node-parallel.

Design:
  - Nodes sharded 6250/core (padded to 6272 = 49*128). Per-core nodes are
    degree-sorted so 128-node dst tiles are degree-homogeneous.
  - Per layer: h^T = W @ x^T on PE (f32r); h rows (bf16) via PE transpose;
    AllGather h rows -> replicated 50176-row bf16 table in DRAM.
  - Edge aggregation in degree-padded layout: dst-slot = partition,
    edge-round = column. dma_gather (256B rows) fetches neighbor features
    per ~104-round segment. Attention logits are batched per segment: one
    broadcast DVE multiply + one strided reduce gives all round dots; the
    per-edge weight apply is one broadcast DVE multiply; rounds are
    accumulated with identity-matmuls into PSUM (exact f32).
  - int16 gather indices limit 32767 -> lo/hi split tables with per-dst
    balanced assignment (overlap window makes padding small); slots are
    regrouped by (lo,hi) counts so tiles are homogeneous in both parts
    (round padding ~1.18x).
  - PairNorm stats via tiny AllGather; JK final linear accumulated per layer.
"""

import numpy as np
import ml_dtypes
from contextlib import ExitStack

import concourse.bass as bass
import concourse.bacc as bacc
import concourse.tile as tile
import concourse.mybir as mybir

F32 = mybir.dt.float32
F32R = mybir.dt.float32r
BF16 = mybir.dt.bfloat16
I16 = mybir.dt.int16
AX = mybir.AxisListType
OP = mybir.AluOpType
AF = mybir.ActivationFunctionType

N = 50000
E = 640000
IN = 128
HID = 128
L = 10
NC = 8
NSH = N // NC          # 6250
TILES = 49
NSHP = TILES * 128     # 6272
TBL = NSHP * NC        # 50176
HI_BASE = TBL - 32768  # 17408
NEG = 0.2
PEPS = 1e-5
SEPS = 1e-16
SEG_MAX_ROUNDS = 104


def preprocess(edge_index):
    """Static graph preprocessing. Returns (meta, percore) where meta is
    core-independent structure (round counts, segments) and percore has the
    int16 index arrays + masks per core."""
    src = np.asarray(edge_index[0], dtype=np.int64)
    dst = np.asarray(edge_index[1], dtype=np.int64)
    owner = dst // NSH

    def split_lists(orders):
        """Given per-core slot orders, classify every edge's src row into the
        lo/hi gather windows (balanced per dst slot). Returns per-core lists
        plus per-core per-slot (lo, hi) counts."""
        inv_all = np.empty(N, np.int64)   # global node -> slot within owner
        for c in range(NC):
            inv = np.empty(NSH, np.int64)
            inv[orders[c]] = np.arange(NSH)
            inv_all[c * NSH:(c + 1) * NSH] = inv
        tblrow_of_src = (src // NSH) * NSHP + inv_all[src]
        lo_lists = [[[] for _ in range(NSHP)] for _ in range(NC)]
        hi_lists = [[[] for _ in range(NSHP)] for _ in range(NC)]
        nlo = np.zeros((NC, NSHP), np.int64)
        nhi = np.zeros((NC, NSHP), np.int64)
        for c in range(NC):
            m = owner == c
            rows = tblrow_of_src[m]
            dpos = inv_all[dst[m]]
            o = np.argsort(dpos, kind="stable")
            rows = rows[o]
            dpos = dpos[o]
            counts = np.bincount(dpos, minlength=NSH)
            starts = np.concatenate([[0], np.cumsum(counts)])
            for p in range(NSH):
                r = rows[starts[p]:starts[p + 1]]
                ml = r[r < HI_BASE]
                mh = r[r > 32767]
                fx = r[(r >= HI_BASE) & (r <= 32767)]
                nl, nh = len(ml), len(mh)
                lo_e, hi_e = [], []
                for v in fx:
                    if nl <= nh:
                        lo_e.append(v); nl += 1
                    else:
                        hi_e.append(v); nh += 1
                lo_lists[c][p] = np.concatenate([ml, np.array(lo_e, np.int64)]) if (len(ml) + len(lo_e)) else np.empty(0, np.int64)
                hi_lists[c][p] = np.concatenate([mh, np.array(hi_e, np.int64)]) if (len(mh) + len(hi_e)) else np.empty(0, np.int64)
                nlo[c, p], nhi[c, p] = nl, nh
        return lo_lists, hi_lists, nlo, nhi

    # pass 1: degree-sorted slots
    orders = []
    for c in range(NC):
        m = owner == c
        dloc = dst[m] - c * NSH
        deg = np.bincount(dloc, minlength=NSH)
        orders.append(np.argsort(-deg, kind="stable"))
    _, _, nlo, nhi = split_lists(orders)

    # passes 2-4: regroup slots by (lo desc, hi desc) so 128-slot tiles are
    # homogeneous in both gather parts (cuts round padding ~25%); the split
    # depends on the order for boundary cores, so iterate to convergence.
    for _ in range(3):
        for c in range(NC):
            perm2 = np.lexsort((-nhi[c, :NSH], -nlo[c, :NSH]))
            orders[c] = orders[c][perm2]
        lo_lists, hi_lists, nlo, nhi = split_lists(orders)

    # common round structure (max over cores)
    D_lo = np.zeros(TILES, np.int64)
    D_hi = np.zeros(TILES, np.int64)
    for t in range(TILES):
        for c in range(NC):
            for sl in range(128):
                p = t * 128 + sl
                D_lo[t] = max(D_lo[t], len(lo_lists[c][p]))
                D_hi[t] = max(D_hi[t], len(hi_lists[c][p]))
    rounds_tot = int((D_lo + D_hi).sum())

    # segments: greedy group tiles
    segs = []
    cur = []
    cur_r = 0
    for t in range(TILES):
        rt = int(D_lo[t] + D_hi[t])
        if cur and cur_r + rt > SEG_MAX_ROUNDS:
            segs.append(cur)
            cur, cur_r = [], 0
        cur.append(t)
        cur_r += rt
    if cur:
        segs.append(cur)

    # build per-core idx arrays (wrapped int16) and masks
    # idx col layout: for each seg: [lo rounds (tile-major)] then [hi rounds]
    # mask col layout: tile-major, per tile lo rounds then hi rounds
    def wrap_idx(flat):
        n = len(flat)
        assert n % 16 == 0
        w = np.asarray(flat, np.int16).reshape(-1, 16).T  # [16, n/16]
        return np.tile(w, (8, 1))                          # [128, n/16]

    percore = []
    for c in range(NC):
        idx_blocks = []
        # mask columns follow the gather-buffer order: per segment, the lo
        # rounds (tile-major) then the hi rounds (tile-major).
        mask = np.zeros((128, rounds_tot), np.float32)
        gcol = 0
        for seg in segs:
            for part, D in (("lo", D_lo), ("hi", D_hi)):
                flat = []
                for t in seg:
                    lists = lo_lists[c] if part == "lo" else hi_lists[c]
                    base = 0 if part == "lo" else HI_BASE
                    for k in range(int(D[t])):
                        for sl in range(128):
                            p = t * 128 + sl
                            lst = lists[p]
                            if k < len(lst):
                                flat.append(int(lst[k]) - base)
                                mask[sl, gcol] = 1.0
                            else:
                                flat.append(0)
                        gcol += 1
                if flat:
                    idx_blocks.append(wrap_idx(flat))
        idx_all = np.concatenate(idx_blocks, axis=1) if idx_blocks else np.zeros((128, 1), np.int16)
        percore.append({"idx": idx_all, "mask": mask, "order": orders[c]})

    meta = {"D_lo": D_lo, "D_hi": D_hi, "segs": segs, "rounds_tot": rounds_tot,
            "dmax": int((D_lo + D_hi).max())}
    pad_eff = 128 * rounds_tot * NC / E
    meta["pad_eff"] = pad_eff
    return meta, percore


def build(nc, meta, n_layers=L, sim_safe=False, do_gather=True, do_rounds=True,
          do_collective=True, do_stats=True, single_packet=False, jb_bufs=1):
    """Emit the full Bass program under a TileContext."""
    D_lo, D_hi, segs = meta["D_lo"], meta["D_hi"], meta["segs"]
    rounds_tot = meta["rounds_tot"]
    MAXNT = max(len(s) for s in segs)
    GELU = AF.Sigmoid if sim_safe else AF.Gelu

    # ---- DRAM tensors
    xT_in = nc.dram_tensor("xT", [128, NSHP], F32R, kind="ExternalInput")
    idx_in = nc.dram_tensor("idx", [128, 8 * rounds_tot], I16, kind="ExternalInput")
    mask_in = nc.dram_tensor("mask", [128, rounds_tot], BF16, kind="ExternalInput")
    W_in = nc.dram_tensor("Wst", [n_layers, 128, 128], F32R, kind="ExternalInput")
    asrc_in = nc.dram_tensor("asrc", [n_layers, 128, 128], BF16, kind="ExternalInput")
    adst_in = nc.dram_tensor("adst", [n_layers, 128, 128], BF16, kind="ExternalInput")
    biasr_in = nc.dram_tensor("biasr", [n_layers, 128, 128], F32, kind="ExternalInput")
    linw_in = nc.dram_tensor("linw", [n_layers, 128, 128], F32R, kind="ExternalInput")
    linbr_in = nc.dram_tensor("linbr", [128, 128], F32, kind="ExternalInput")
    identb_in = nc.dram_tensor("identb", [128, 128], BF16, kind="ExternalInput")
    identf_in = nc.dram_tensor("identf", [128, 128], F32, kind="ExternalInput")
    ones_in = nc.dram_tensor("ones", [128, 128], F32, kind="ExternalInput")
    padm_in = nc.dram_tensor("padm", [128, 1], F32, kind="ExternalInput")
    y_out = nc.dram_tensor("y", [NSHP, 128], F32, kind="ExternalOutput")

    ag_in = nc.dram_tensor("ag_in", [NSHP, 128], BF16)
    table = nc.dram_tensor("table", [TBL, 128], BF16, addr_space="Shared")
    st_in = nc.dram_tensor("st_in", [128, 2], F32)
    st_out = nc.dram_tensor("st_out", [NC * 128, 2], F32, addr_space="Shared")

    RG = [list(range(NC))]

    with tile.TileContext(nc) as tc, ExitStack() as ctx:
        # ---- persistent SBUF (bufs=1 pools)
        P = ctx.enter_context(tc.tile_pool(name="persist", bufs=1))
        xT = P.tile([128, NSHP], F32R, tag="xT")
        hTb = P.tile([128, NSHP], BF16, tag="hTb")
        hrows = P.tile([128, NSHP], BF16, tag="hrows")
        outfin = P.tile([128, NSHP], F32, tag="outfin")
        idx_sb = P.tile([128, 8 * rounds_tot], I16, tag="idx")
        mask_sb = P.tile([128, rounds_tot], BF16, tag="mask")
        W_sb = P.tile([128, n_layers * 128], F32R, tag="W")
        asrc_sb = P.tile([128, n_layers * 128], BF16, tag="asrc")
        adst_sb = P.tile([128, n_layers * 128], BF16, tag="adst")
        biasr_sb = P.tile([128, n_layers * 128], F32, tag="biasr")
        linw_sb = P.tile([128, n_layers * 128], F32R, tag="linw")
        linbr_sb = P.tile([128, 128], F32, tag="linbr")
        identb = P.tile([128, 128], BF16, tag="identb")
        identf = P.tile([128, 128], F32, tag="identf")
        ones_sb = P.tile([128, 128], F32, tag="ones")
        padm_sb = P.tile([128, 1], F32, tag="padm")
        as_own = P.tile([128, TILES], F32, tag="as_own")
        ad_own = P.tile([128, TILES], F32, tag="ad_own")
        wself = P.tile([128, TILES], F32, tag="wself")
        stp = P.tile([128, 2], F32, tag="stp")
        gtmp = P.tile([128, 16], F32, tag="gtmp")
        adx = P.tile([128, rounds_tot], F32, tag="adx")
        gstat = P.tile([128, 2], F32, tag="gstat")

        # ---- input loads
        nc.sync.dma_start(idx_sb[:], idx_in.ap())
        nc.sync.dma_start(mask_sb[:], mask_in.ap())
        nc.sync.dma_start(W_sb[:].rearrange("a (l b) -> a l b", b=128), W_in.ap().rearrange("l a b -> a l b"))
        nc.sync.dma_start(asrc_sb[:].rearrange("a (l b) -> a l b", b=128), asrc_in.ap().rearrange("l a b -> a l b"))
        nc.sync.dma_start(adst_sb[:].rearrange("a (l b) -> a l b", b=128), adst_in.ap().rearrange("l a b -> a l b"))
        nc.sync.dma_start(biasr_sb[:].rearrange("a (l b) -> a l b", b=128), biasr_in.ap().rearrange("l a b -> a l b"))
        nc.sync.dma_start(linw_sb[:].rearrange("a (l b) -> a l b", b=128), linw_in.ap().rearrange("l a b -> a l b"))
        nc.sync.dma_start(linbr_sb[:], linbr_in.ap())
        nc.sync.dma_start(identb[:], identb_in.ap())
        nc.sync.dma_start(identf[:], identf_in.ap())
        nc.sync.dma_start(ones_sb[:], ones_in.ap())
        nc.sync.dma_start(padm_sb[:], padm_in.ap())
        nc.sync.dma_start(xT[:], xT_in.ap())

        # ---- working pools
        PD = ctx.enter_context(tc.tile_pool(name="pd", bufs=2, space="PSUM"))
        PT = ctx.enter_context(tc.tile_pool(name="pt", bufs=2, space="PSUM"))
        PA = ctx.enter_context(tc.tile_pool(name="pa", bufs=2, space="PSUM"))
        PM = ctx.enter_context(tc.tile_pool(name="pm", bufs=2, space="PSUM"))
        GSEG = ctx.enter_context(tc.tile_pool(name="gseg", bufs=2))
        JB = ctx.enter_context(tc.tile_pool(name="jb", bufs=jb_bufs))
        SC = ctx.enter_context(tc.tile_pool(name="sc", bufs=2))
        SS = ctx.enter_context(tc.tile_pool(name="ss", bufs=2))
        RP = ctx.enter_context(tc.tile_pool(name="rp", bufs=1))
        EP = ctx.enter_context(tc.tile_pool(name="ep", bufs=1))

        # idx col offsets (wrapped layout: 8 cols per round)
        idx_off = {}
        ic = 0
        for si, seg in enumerate(segs):
            lo_r = int(sum(D_lo[t] for t in seg))
            hi_r = int(sum(D_hi[t] for t in seg))
            idx_off[si] = (ic, lo_r, ic + 8 * lo_r, hi_r)
            ic += 8 * (lo_r + hi_r)

        for l in range(n_layers):
            wsl = W_sb[:, l * 128:(l + 1) * 128]
            asl = asrc_sb[:, l * 128:(l + 1) * 128]
            adl = adst_sb[:, l * 128:(l + 1) * 128]
            bsl = biasr_sb[:, l * 128:(l + 1) * 128]
            lwl = linw_sb[:, l * 128:(l + 1) * 128]

            # ---- dense: hT = W.T @ xT  (psum chunks of 512)
            for ch0 in range(0, NSHP, 512):
                chsz = min(512, NSHP - ch0)
                pd = PD.tile([128, 512], F32, tag="pd")
                nc.tensor.matmul(pd[:, :chsz], wsl, xT[:, ch0:ch0 + chsz],
                                 start=True, stop=True)
                nc.vector.tensor_copy(hTb[:, ch0:ch0 + chsz], pd[:, :chsz])

            # ---- transpose hT -> h rows (bf16)
            for t in range(TILES):
                sl = slice(t * 128, (t + 1) * 128)
                pt = PT.tile([128, 128], BF16, tag="pt")
                nc.tensor.transpose(pt[:], hTb[:, sl], identb[:])
                nc.vector.tensor_copy(hrows[:, sl], pt[:])

            # ---- AllGather h rows
            if do_collective:
                nc.sync.dma_start(ag_in.ap().rearrange("(t p) f -> p t f", p=128),
                                  hrows[:].rearrange("p (t f) -> p t f", f=128))
                nc.gpsimd.collective_compute(
                    "AllGather", OP.bypass, replica_groups=RG,
                    ins=[ag_in.ap()], outs=[table.ap()])

            # ---- own alpha dots (batched over all tiles) + self weight
            hr3 = hrows[:].rearrange("p (t f) -> p t f", f=128)
            sc3 = hTb[:].rearrange("p (t f) -> p t f", f=128)
            asl3 = asl.rearrange("p (o f) -> p o f", o=1)
            adl3 = adl.rearrange("p (o f) -> p o f", o=1)
            i0, i1 = bass.broadcast_tensor_aps(hr3, asl3)
            nc.vector.tensor_tensor(sc3, i0, i1, op=OP.mult)
            nc.vector.reduce_sum(out=as_own[:], in_=sc3, axis=AX.X)
            i0, i1 = bass.broadcast_tensor_aps(hr3, adl3)
            nc.vector.tensor_tensor(sc3, i0, i1, op=OP.mult)
            nc.vector.reduce_sum(out=ad_own[:], in_=sc3, axis=AX.X)
            zs = SC.tile([128, TILES], F32, tag="zself")
            nc.vector.tensor_tensor(zs[:], as_own[:], ad_own[:], op=OP.add)
            zs2 = SC.tile([128, TILES], F32, tag="zself")
            nc.vector.scalar_tensor_tensor(
                out=zs2[:], in0=zs[:], scalar=NEG, in1=zs[:], op0=OP.mult, op1=OP.max)
            nc.scalar.activation(wself[:], zs2[:], AF.Exp)

            # ---- expand ad_own into gather-column order on the idle ACT
            # engine (per-partition bias broadcast), so the per-segment z2
            # add is one batched DVE op per half instead of 2 ops per tile.
            acol = 0
            for seg_ in segs:
                for D_ in (D_lo, D_hi):
                    for t_ in seg_:
                        d_ = int(D_[t_])
                        nc.scalar.activation(adx[:, acol:acol + d_],
                                             ones_sb[:, :d_], AF.Copy,
                                             scale=ad_own[:, t_:t_ + 1])
                        acol += d_

            # ---- aggregation over segments
            seg_base = 0
            for si, seg in enumerate(segs):
                lo_r = int(sum(D_lo[t] for t in seg))
                hi_r = int(sum(D_hi[t] for t in seg))
                seg_r = lo_r + hi_r
                sb0 = seg_base
                seg_base += seg_r
                gbuf = None
                if do_gather or do_rounds:
                    gbuf = GSEG.tile([128, SEG_MAX_ROUNDS * 128], BF16, tag="gseg")
                    g3 = gbuf[:].rearrange("p (r e) -> p r e", e=128)
                ic_lo, nlo, ic_hi, nhi = idx_off[si]
                if not do_gather and do_rounds:
                    nc.vector.memset(gbuf[:, :seg_r * 128], 0.0)
                if lo_r and do_gather:
                    nc.gpsimd.dma_gather(
                        g3[:, :lo_r, :], table.ap()[:32768, :],
                        idx_sb[:, ic_lo:ic_lo + 8 * lo_r],
                        128 * lo_r, 128 * lo_r, 128, single_packet=single_packet)
                if hi_r and do_gather:
                    nc.gpsimd.dma_gather(
                        g3[:, lo_r:seg_r, :], table.ap()[HI_BASE:TBL, :],
                        idx_sb[:, ic_hi:ic_hi + 8 * hi_r],
                        128 * hi_r, 128 * hi_r, 128, single_packet=single_packet)

                if not do_rounds:
                    for t in seg:
                        tsl = slice(t * 128, (t + 1) * 128)
                        row = RP.tile([128, 128], F32, tag="row")
                        nc.vector.tensor_copy(row[:], hrows[:, tsl])
                        ptf = PM.tile([128, 128], F32, tag="pm")
                        nc.tensor.transpose(ptf[:], row[:], identf[:])
                        nc.vector.tensor_copy(xT[:, tsl], ptf[:])
                    continue

                # ---- batched attention logits, split lo/hi so the lo-half
                # chain overlaps the hi gather (subtile deps give the early
                # start: the lo ops read only the lo region of gbuf).
                jb = JB.tile([128, SEG_MAX_ROUNDS * 128], BF16, tag="jb")
                z = SC.tile([128, SEG_MAX_ROUNDS], F32, tag="z")
                z2 = SC.tile([128, SEG_MAX_ROUNDS], F32, tag="z2")
                zl = SC.tile([128, SEG_MAX_ROUNDS], F32, tag="zl")
                ew = SC.tile([128, SEG_MAX_ROUNDS], F32, tag="ew")
                ewm = SC.tile([128, SEG_MAX_ROUNDS], F32, tag="ewm")
                for r0, r1 in ((0, lo_r), (lo_r, seg_r)):
                    if r0 == r1:
                        continue
                    gp = gbuf[:, r0 * 128:r1 * 128].rearrange(
                        "p (r e) -> p r e", e=128)
                    jp = jb[:, r0 * 128:r1 * 128].rearrange(
                        "p (r e) -> p r e", e=128)
                    i0, i1 = bass.broadcast_tensor_aps(gp, asl3)
                    nc.vector.tensor_tensor(jp, i0, i1, op=OP.mult)
                    nc.vector.reduce_sum(out=z[:, r0:r1], in_=jp, axis=AX.X)
                    nc.vector.tensor_tensor(
                        z2[:, r0:r1], z[:, r0:r1],
                        adx[:, sb0 + r0:sb0 + r1], op=OP.add)
                    nc.vector.scalar_tensor_tensor(
                        out=zl[:, r0:r1], in0=z2[:, r0:r1], scalar=NEG,
                        in1=z2[:, r0:r1], op0=OP.mult, op1=OP.max)
                    nc.scalar.activation(ew[:, r0:r1], zl[:, r0:r1], AF.Exp)
                    nc.vector.tensor_tensor(
                        ewm[:, r0:r1], ew[:, r0:r1],
                        mask_sb[:, sb0 + r0:sb0 + r1], op=OP.mult)
                    # weight apply for this half: jb <- gbuf * ewm (broadcast)
                    ep = ewm[:, r0:r1].rearrange("p (r o) -> p r o", o=1)
                    i0, i1 = bass.broadcast_tensor_aps(gp, ep)
                    nc.vector.tensor_tensor(jp, i0, i1, op=OP.mult)

                # ---- per-tile accumulate; epilogue math batched per segment
                n_t = len(seg)
                t0 = seg[0]
                s1g = SC.tile([128, MAXNT], F32, tag="s1g")
                s2g = SC.tile([128, MAXNT], F32, tag="s2g")
                paseg = EP.tile([128, MAXNT * 128], F32, tag="paseg")
                lo_c = 0
                hi_c = 0
                for ti, t in enumerate(seg):
                    dlo, dhi = int(D_lo[t]), int(D_hi[t])
                    Dt = dlo + dhi
                    nc.vector.reduce_sum(out=s1g[:, ti:ti + 1],
                                         in_=ewm[:, lo_c:lo_c + dlo], axis=AX.X)
                    nc.vector.reduce_sum(
                        out=s2g[:, ti:ti + 1],
                        in_=ewm[:, lo_r + hi_c:lo_r + hi_c + dhi], axis=AX.X)
                    pa = PA.tile([128, 128], F32, tag="pa")
                    for k in range(Dt):
                        c = (lo_c + k) if k < dlo else (lo_r + hi_c + k - dlo)
                        nc.tensor.matmul(pa[:], identb[:],
                                         jb[:, c * 128:(c + 1) * 128],
                                         start=(k == 0), stop=(k == Dt - 1))
                    nc.vector.tensor_copy(paseg[:, ti * 128:(ti + 1) * 128], pa[:])
                    lo_c += dlo
                    hi_c += dhi

                # softmax denominators for all tiles of the segment at once
                spg = SC.tile([128, MAXNT], F32, tag="spg")
                nc.vector.scalar_tensor_tensor(
                    out=spg[:, :n_t], in0=s1g[:, :n_t], scalar=SEPS,
                    in1=s2g[:, :n_t], op0=OP.add, op1=OP.add)
                sp2g = SC.tile([128, MAXNT], F32, tag="sp2g")
                nc.vector.tensor_tensor(sp2g[:, :n_t], spg[:, :n_t],
                                        wself[:, t0:t0 + n_t], op=OP.add)
                recg = SC.tile([128, MAXNT], F32, tag="recg")
                nc.vector.reciprocal(recg[:, :n_t], sp2g[:, :n_t])

                # acc = hrows*wself + pa ; row = acc*rec + bias  (3D bcast)
                ssl = slice(t0 * 128, (t0 + n_t) * 128)
                hr3s = hrows[:, ssl].rearrange("p (t f) -> p t f", f=128)
                ws3 = wself[:, t0:t0 + n_t].rearrange("p (t o) -> p t o", o=1)
                acc1g = EP.tile([128, MAXNT * 128], F32, tag="acc1g")
                a3 = acc1g[:, :n_t * 128].rearrange("p (t f) -> p t f", f=128)
                i0, i1 = bass.broadcast_tensor_aps(hr3s, ws3)
                nc.vector.tensor_tensor(a3, i0, i1, op=OP.mult)
                rowg = EP.tile([128, MAXNT * 128], F32, tag="rowg")
                nc.vector.tensor_tensor(rowg[:, :n_t * 128], acc1g[:, :n_t * 128],
                                        paseg[:, :n_t * 128], op=OP.add)
                r3 = rowg[:, :n_t * 128].rearrange("p (t f) -> p t f", f=128)
                rec3 = recg[:, :n_t].rearrange("p (t o) -> p t o", o=1)
                i0, i1 = bass.broadcast_tensor_aps(r3, rec3)
                nc.vector.tensor_tensor(a3, i0, i1, op=OP.mult)
                b3 = bsl.rearrange("p (o f) -> p o f", o=1)
                i0, i1 = bass.broadcast_tensor_aps(a3, b3)
                nc.vector.tensor_tensor(r3, i0, i1, op=OP.add)
                if seg[-1] == TILES - 1:
                    lsl = slice((n_t - 1) * 128, n_t * 128)
                    nc.vector.tensor_scalar_mul(rowg[:, lsl], rowg[:, lsl],
                                                padm_sb[:])
                # transpose pre-pairnorm rows into xT slab
                for ti, t in enumerate(seg):
                    ptf = PM.tile([128, 128], F32, tag="pm")
                    nc.tensor.transpose(ptf[:],
                                        rowg[:, ti * 128:(ti + 1) * 128],
                                        identf[:])
                    nc.vector.tensor_copy(xT[:, t * 128:(t + 1) * 128], ptf[:])

            # ---- pairnorm stats (on transposed pre-pairnorm slab)
            fsum = SS.tile([128, 1], F32, tag="fsum")
            nc.vector.reduce_sum(out=fsum[:], in_=xT[:], axis=AX.X)
            sqc = SS.tile([128, 1], F32, tag="sqc")
            nc.scalar.activation(hTb[:], xT[:], AF.Square, accum_out=sqc[:])
            nc.vector.tensor_copy(stp[:, :1], fsum[:])
            nc.vector.tensor_copy(stp[:, 1:2], sqc[:])
            if do_stats:
                nc.sync.dma_start(st_in.ap(), stp[:])
                nc.gpsimd.collective_compute(
                    "AllGather", OP.bypass, replica_groups=RG,
                    ins=[st_in.ap()], outs=[st_out.ap()])
                nc.sync.dma_start(gtmp[:].rearrange("p (r c) -> p r c", c=2),
                                  st_out.ap().rearrange("(r p) c -> p r c", p=128))
                nc.vector.reduce_sum(out=gstat[:],
                                     in_=gtmp[:].rearrange("p (r c) -> p c r", c=2),
                                     axis=AX.X)
            else:
                nc.vector.tensor_scalar_mul(gstat[:], stp[:], float(NC))
            mu = SS.tile([128, 1], F32, tag="mu")
            nc.vector.tensor_scalar_mul(mu[:], gstat[:, :1], 1.0 / N)
            st2 = SS.tile([128, 2], F32, tag="st2")
            nc.vector.tensor_copy(st2[:, :1], gstat[:, 1:2])
            nc.vector.tensor_tensor(st2[:, 1:2], mu[:], mu[:], op=OP.mult)
            p2 = PM.tile([128, 128], F32, tag="pm")
            nc.tensor.matmul(p2[:1, :2], ones_sb[:, :1], st2[:],
                             start=True, stop=True)
            tot = SS.tile([1, 2], F32, tag="tot")
            nc.vector.tensor_copy(tot[:], p2[:1, :2])
            v3 = SS.tile([1, 1], F32, tag="v3")
            nc.vector.tensor_scalar(v3[:], tot[:, :1], 1.0 / N, PEPS,
                                    op0=OP.mult, op1=OP.add)
            v4 = SS.tile([1, 1], F32, tag="v4")
            nc.vector.tensor_tensor(v4[:], v3[:], tot[:, 1:2], op=OP.subtract)
            den = SS.tile([1, 1], F32, tag="den")
            nc.scalar.activation(den[:], v4[:], AF.Sqrt)
            invd = SS.tile([1, 1], F32, tag="invd")
            nc.vector.reciprocal(invd[:], den[:])
            pb1 = PM.tile([128, 128], F32, tag="pm")
            nc.tensor.matmul(pb1[:, :1], ones_sb[:1, :], invd[:],
                             start=True, stop=True)
            invdr = SS.tile([128, 1], F32, tag="invdr")
            nc.vector.tensor_copy(invdr[:], pb1[:, :1])
            nms = SS.tile([128, 1], F32, tag="nms")
            nc.vector.tensor_scalar(nms[:], mu[:], invdr[:], -1.0,
                                    op0=OP.mult, op1=OP.mult)

            # ---- pairnorm + gelu fused on ACT (in-place on xT)
            for ch0 in range(0, NSHP, 512):
                chsz = min(512, NSHP - ch0)
                csl = slice(ch0, ch0 + chsz)
                nc.scalar.activation(xT[:, csl], xT[:, csl], GELU,
                                     bias=nms[:], scale=invdr[:])

            # ---- final-linear increment
            for t in range(TILES):
                tsl = slice(t * 128, (t + 1) * 128)
                pf = PM.tile([128, 128], F32, tag="pm")
                nc.tensor.matmul(pf[:], xT[:, tsl], lwl, start=True, stop=True)
                if l == 0:
                    nc.vector.scalar_tensor_tensor(
                        out=outfin[:, tsl], in0=pf[:], scalar=1.0, in1=linbr_sb[:],
                        op0=OP.mult, op1=OP.add)
                else:
                    nc.vector.scalar_tensor_tensor(
                        out=outfin[:, tsl], in0=pf[:], scalar=1.0,
                        in1=outfin[:, tsl], op0=OP.mult, op1=OP.add)

        # ---- write final output
        nc.sync.dma_start(y_out.ap().rearrange("(t p) f -> p t f", p=128),
                          outfin[:].rearrange("p (t f) -> p t f", f=128))

    return nc


def make_inputs(inputs, meta, percore, n_layers=L):
    """Build per-core in_maps from the full problem inputs."""
    x = np.asarray(inputs["x"], np.float32)
    W0 = np.asarray(inputs["W0"], np.float32)
    Ws = np.asarray(inputs["Ws"], np.float32)
    att_src = np.asarray(inputs["att_src"], np.float32)
    att_dst = np.asarray(inputs["att_dst"], np.float32)
    bias = np.asarray(inputs["bias"], np.float32)
    lin_w = np.asarray(inputs["lin_w"], np.float32)
    lin_b = np.asarray(inputs["lin_b"], np.float32)

    Wst = np.stack([W0] + [Ws[i] for i in range(n_layers - 1)]).astype(np.float32)
    asrc = np.stack([np.tile(att_src[i], (128, 1)) for i in range(n_layers)]).astype(ml_dtypes.bfloat16)
    adst = np.stack([np.tile(att_dst[i], (128, 1)) for i in range(n_layers)]).astype(ml_dtypes.bfloat16)
    biasr = np.stack([np.tile(bias[i], (128, 1)) for i in range(n_layers)]).astype(np.float32)
    linw = np.stack([lin_w[i * HID:(i + 1) * HID] for i in range(n_layers)]).astype(np.float32)
    linbr = np.tile(lin_b, (128, 1)).astype(np.float32)
    identb = np.eye(128, dtype=ml_dtypes.bfloat16)
    identf = np.eye(128, dtype=np.float32)
    ones = np.ones((128, 128), np.float32)
    padm = np.zeros((128, 1), np.float32)
    padm[:NSH - (TILES - 1) * 128] = 1.0

    in_maps = []
    for c in range(NC):
        order = percore[c]["order"]
        xs = x[c * NSH:(c + 1) * NSH][order]                       # [6250,128] sorted
        xT = np.zeros((128, NSHP), np.float32)
        xT[:, :NSH] = xs.T
        in_maps.append({
            "xT": xT, "idx": percore[c]["idx"], "mask": percore[c]["mask"].astype(ml_dtypes.bfloat16),
            "Wst": Wst, "asrc": asrc, "adst": adst, "biasr": biasr,
            "linw": linw, "linbr": linbr, "identb": identb, "identf": identf,
            "ones": ones, "padm": padm,
        })
    return in_maps


def assemble_output(results, percore):
    """Concatenate per-core outputs, undoing the degree-sort permutation."""
    out = np.empty((N, HID), np.float32)
    for c in range(NC):
        order = percore[c]["order"]
        yc = results[c]["y"][:NSH]     # sorted order
        out[c * NSH + order] = yc
    return out


# ---------------------------------------------------------------------------
# kernel() entry point
# ---------------------------------------------------------------------------
_CACHE = {}


def _get_compiled(edge_key, edge_index):
    if edge_key not in _CACHE:
        meta, percore = preprocess(edge_index)
        nc = bacc.Bacc("TRN2", target_bir_lowering=False, debug=False,
                       num_devices=NC)
        build(nc, meta, n_layers=L, sim_safe=False)
        nc.compile()
        _CACHE[edge_key] = (nc, meta, percore)
    return _CACHE[edge_key]


def kernel(**inputs):
    from concourse.bass_utils import run_bass_kernel_spmd
    edge_index = np.asarray(inputs["edge_index"])
    edge_key = hash(edge_index.tobytes())
    nc, meta, percore = _get_compiled(edge_key, edge_index)
    in_maps = make_inputs(inputs, meta, percore, n_layers=L)
    res = run_bass_kernel_spmd(nc, in_maps, list(range(NC)))
    return assemble_output(res.results, percore)



# revision 44
# speedup vs baseline: 1.0435x; 1.0435x over previous
"""GAT encoder (10-layer, JK-concat) Trainium2 Bass kernel — 8-core node-parallel.

Design:
  - Nodes sharded 6250/core (padded to 6272 = 49*128). Per-core nodes are
    degree-sorted so 128-node dst tiles are degree-homogeneous.
  - Per layer: h^T = W @ x^T on PE (f32r); h rows (bf16) via PE transpose;
    AllGather h rows -> replicated 50176-row bf16 table in DRAM.
  - Edge aggregation in degree-padded layout: dst-slot = partition,
    edge-round = column. dma_gather (256B rows) fetches neighbor features
    per ~104-round segment. Attention logits are batched per segment: one
    broadcast DVE multiply + one strided reduce gives all round dots; the
    per-edge weight apply is one broadcast DVE multiply; rounds are
    accumulated with identity-matmuls into PSUM (exact f32).
  - int16 gather indices limit 32767 -> lo/hi split tables with per-dst
    balanced assignment (overlap window makes padding small); slots are
    regrouped by (lo,hi) counts so tiles are homogeneous in both parts
    (round padding ~1.18x).
  - PairNorm stats via tiny AllGather; JK final linear accumulated per layer.
"""

import numpy as np
import ml_dtypes
from contextlib import ExitStack

import concourse.bass as bass
import concourse.bacc as bacc
import concourse.tile as tile
import concourse.mybir as mybir

F32 = mybir.dt.float32
F32R = mybir.dt.float32r
BF16 = mybir.dt.bfloat16
I16 = mybir.dt.int16
AX = mybir.AxisListType
OP = mybir.AluOpType
AF = mybir.ActivationFunctionType

N = 50000
E = 640000
IN = 128
HID = 128
L = 10
NC = 8
NSH = N // NC          # 6250
TILES = 49
NSHP = TILES * 128     # 6272
TBL = NSHP * NC        # 50176
HI_BASE = TBL - 32768  # 17408
NEG = 0.2
PEPS = 1e-5
SEPS = 1e-16
SEG_MAX_ROUNDS = 104


def preprocess(edge_index):
    """Static graph preprocessing. Returns (meta, percore) where meta is
    core-independent structure (round counts, segments) and percore has the
    int16 index arrays + masks per core."""
    src = np.asarray(edge_index[0], dtype=np.int64)
    dst = np.asarray(edge_index[1], dtype=np.int64)
    owner = dst // NSH

    def split_lists(orders):
        """Given per-core slot orders, classify every edge's src row into the
        lo/hi gather windows (balanced per dst slot). Returns per-core lists
        plus per-core per-slot (lo, hi) counts."""
        inv_all = np.empty(N, np.int64)   # global node -> slot within owner
        for c in range(NC):
            inv = np.empty(NSH, np.int64)
            inv[orders[c]] = np.arange(NSH)
            inv_all[c * NSH:(c + 1) * NSH] = inv
        tblrow_of_src = (src // NSH) * NSHP + inv_all[src]
        lo_lists = [[[] for _ in range(NSHP)] for _ in range(NC)]
        hi_lists = [[[] for _ in range(NSHP)] for _ in range(NC)]
        nlo = np.zeros((NC, NSHP), np.int64)
        nhi = np.zeros((NC, NSHP), np.int64)
        for c in range(NC):
            m = owner == c
            rows = tblrow_of_src[m]
            dpos = inv_all[dst[m]]
            o = np.argsort(dpos, kind="stable")
            rows = rows[o]
            dpos = dpos[o]
            counts = np.bincount(dpos, minlength=NSH)
            starts = np.concatenate([[0], np.cumsum(counts)])
            for p in range(NSH):
                r = rows[starts[p]:starts[p + 1]]
                ml = r[r < HI_BASE]
                mh = r[r > 32767]
                fx = r[(r >= HI_BASE) & (r <= 32767)]
                nl, nh = len(ml), len(mh)
                lo_e, hi_e = [], []
                for v in fx:
                    if nl <= nh:
                        lo_e.append(v); nl += 1
                    else:
                        hi_e.append(v); nh += 1
                lo_lists[c][p] = np.concatenate([ml, np.array(lo_e, np.int64)]) if (len(ml) + len(lo_e)) else np.empty(0, np.int64)
                hi_lists[c][p] = np.concatenate([mh, np.array(hi_e, np.int64)]) if (len(mh) + len(hi_e)) else np.empty(0, np.int64)
                nlo[c, p], nhi[c, p] = nl, nh
        return lo_lists, hi_lists, nlo, nhi

    # pass 1: degree-sorted slots
    orders = []
    for c in range(NC):
        m = owner == c
        dloc = dst[m] - c * NSH
        deg = np.bincount(dloc, minlength=NSH)
        orders.append(np.argsort(-deg, kind="stable"))
    _, _, nlo, nhi = split_lists(orders)

    # passes 2-4: regroup slots by (lo desc, hi desc) so 128-slot tiles are
    # homogeneous in both gather parts (cuts round padding ~25%); the split
    # depends on the order for boundary cores, so iterate to convergence.
    for _ in range(3):
        for c in range(NC):
            perm2 = np.lexsort((-nhi[c, :NSH], -nlo[c, :NSH]))
            orders[c] = orders[c][perm2]
        lo_lists, hi_lists, nlo, nhi = split_lists(orders)

    # common round structure (max over cores)
    D_lo = np.zeros(TILES, np.int64)
    D_hi = np.zeros(TILES, np.int64)
    for t in range(TILES):
        for c in range(NC):
            for sl in range(128):
                p = t * 128 + sl
                D_lo[t] = max(D_lo[t], len(lo_lists[c][p]))
                D_hi[t] = max(D_hi[t], len(hi_lists[c][p]))
    rounds_tot = int((D_lo + D_hi).sum())

    # segments: greedy group tiles
    segs = []
    cur = []
    cur_r = 0
    for t in range(TILES):
        rt = int(D_lo[t] + D_hi[t])
        if cur and cur_r + rt > SEG_MAX_ROUNDS:
            segs.append(cur)
            cur, cur_r = [], 0
        cur.append(t)
        cur_r += rt
    if cur:
        segs.append(cur)

    # build per-core idx arrays (wrapped int16) and masks
    # idx col layout: for each seg: [lo rounds (tile-major)] then [hi rounds]
    # mask col layout: tile-major, per tile lo rounds then hi rounds
    def wrap_idx(flat):
        n = len(flat)
        assert n % 16 == 0
        w = np.asarray(flat, np.int16).reshape(-1, 16).T  # [16, n/16]
        return np.tile(w, (8, 1))                          # [128, n/16]

    percore = []
    for c in range(NC):
        idx_blocks = []
        # mask columns follow the gather-buffer order: per segment, the lo
        # rounds (tile-major) then the hi rounds (tile-major).
        mask = np.zeros((128, rounds_tot), np.float32)
        gcol = 0
        for seg in segs:
            for part, D in (("lo", D_lo), ("hi", D_hi)):
                flat = []
                for t in seg:
                    lists = lo_lists[c] if part == "lo" else hi_lists[c]
                    base = 0 if part == "lo" else HI_BASE
                    for k in range(int(D[t])):
                        for sl in range(128):
                            p = t * 128 + sl
                            lst = lists[p]
                            if k < len(lst):
                                flat.append(int(lst[k]) - base)
                                mask[sl, gcol] = 1.0
                            else:
                                flat.append(0)
                        gcol += 1
                if flat:
                    idx_blocks.append(wrap_idx(flat))
        idx_all = np.concatenate(idx_blocks, axis=1) if idx_blocks else np.zeros((128, 1), np.int16)
        percore.append({"idx": idx_all, "mask": mask, "order": orders[c]})

    meta = {"D_lo": D_lo, "D_hi": D_hi, "segs": segs, "rounds_tot": rounds_tot,
            "dmax": int((D_lo + D_hi).max())}
    pad_eff = 128 * rounds_tot * NC / E
    meta["pad_eff"] = pad_eff
    return meta, percore


def build(nc, meta, n_layers=L, sim_safe=False, do_gather=True, do_rounds=True,
          do_collective=True, do_stats=True, single_packet=False, jb_bufs=1):
    """Emit the full Bass program under a TileContext."""
    D_lo, D_hi, segs = meta["D_lo"], meta["D_hi"], meta["segs"]
    rounds_tot = meta["rounds_tot"]
    MAXNT = max(len(s) for s in segs)
    GELU = AF.Sigmoid if sim_safe else AF.Gelu

    # ---- DRAM tensors
    xT_in = nc.dram_tensor("xT", [128, NSHP], F32R, kind="ExternalInput")
    idx_in = nc.dram_tensor("idx", [128, 8 * rounds_tot], I16, kind="ExternalInput")
    mask_in = nc.dram_tensor("mask", [128, rounds_tot], BF16, kind="ExternalInput")
    W_in = nc.dram_tensor("Wst", [n_layers, 128, 128], F32R, kind="ExternalInput")
    asrc_in = nc.dram_tensor("asrc", [n_layers, 128, 128], BF16, kind="ExternalInput")
    adst_in = nc.dram_tensor("adst", [n_layers, 128, 128], BF16, kind="ExternalInput")
    biasr_in = nc.dram_tensor("biasr", [n_layers, 128, 128], F32, kind="ExternalInput")
    linw_in = nc.dram_tensor("linw", [n_layers, 128, 128], F32R, kind="ExternalInput")
    linbr_in = nc.dram_tensor("linbr", [128, 128], F32, kind="ExternalInput")
    linbc_in = nc.dram_tensor("linbc", [128, 1], F32, kind="ExternalInput")
    identb_in = nc.dram_tensor("identb", [128, 128], BF16, kind="ExternalInput")
    identf_in = nc.dram_tensor("identf", [128, 128], F32, kind="ExternalInput")
    ones_in = nc.dram_tensor("ones", [128, 128], F32, kind="ExternalInput")
    padm_in = nc.dram_tensor("padm", [128, 1], F32, kind="ExternalInput")
    y_out = nc.dram_tensor("y", [NSHP, 128], F32, kind="ExternalOutput")

    ag_in = nc.dram_tensor("ag_in", [NSHP, 128], BF16)
    table = nc.dram_tensor("table", [TBL, 128], BF16, addr_space="Shared")
    st_in = nc.dram_tensor("st_in", [128, 2], F32)
    st_out = nc.dram_tensor("st_out", [NC * 128, 2], F32, addr_space="Shared")

    RG = [list(range(NC))]

    with tile.TileContext(nc) as tc, ExitStack() as ctx:
        # ---- persistent SBUF (bufs=1 pools)
        P = ctx.enter_context(tc.tile_pool(name="persist", bufs=1))
        xT = P.tile([128, NSHP], F32R, tag="xT")
        hTb = P.tile([128, NSHP], BF16, tag="hTb")
        hrows = P.tile([128, NSHP], BF16, tag="hrows")
        outfin = P.tile([128, NSHP], F32, tag="outfin")
        idx_sb = P.tile([128, 8 * rounds_tot], I16, tag="idx")
        mask_sb = P.tile([128, rounds_tot], BF16, tag="mask")
        W_sb = P.tile([128, n_layers * 128], F32R, tag="W")
        asrc_sb = P.tile([128, n_layers * 128], BF16, tag="asrc")
        adst_sb = P.tile([128, n_layers * 128], BF16, tag="adst")
        biasr_sb = P.tile([128, n_layers * 128], F32, tag="biasr")
        linw_sb = P.tile([128, n_layers * 128], F32R, tag="linw")
        linbr_sb = P.tile([128, 128], F32, tag="linbr")
        linbc_sb = P.tile([128, 1], F32, tag="linbc")
        identb = P.tile([128, 128], BF16, tag="identb")
        identf = P.tile([128, 128], F32, tag="identf")
        ones_sb = P.tile([128, 128], F32, tag="ones")
        padm_sb = P.tile([128, 1], F32, tag="padm")
        as_own = P.tile([128, TILES], F32, tag="as_own")
        ad_own = P.tile([128, TILES], F32, tag="ad_own")
        wself = P.tile([128, TILES], F32, tag="wself")
        stp = P.tile([128, 2], F32, tag="stp")
        gtmp = P.tile([128, 16], F32, tag="gtmp")
        adx = P.tile([128, rounds_tot], F32, tag="adx")
        gstat = P.tile([128, 2], F32, tag="gstat")

        # ---- input loads
        nc.sync.dma_start(idx_sb[:], idx_in.ap())
        nc.sync.dma_start(mask_sb[:], mask_in.ap())
        nc.sync.dma_start(W_sb[:].rearrange("a (l b) -> a l b", b=128), W_in.ap().rearrange("l a b -> a l b"))
        nc.sync.dma_start(asrc_sb[:].rearrange("a (l b) -> a l b", b=128), asrc_in.ap().rearrange("l a b -> a l b"))
        nc.sync.dma_start(adst_sb[:].rearrange("a (l b) -> a l b", b=128), adst_in.ap().rearrange("l a b -> a l b"))
        nc.sync.dma_start(biasr_sb[:].rearrange("a (l b) -> a l b", b=128), biasr_in.ap().rearrange("l a b -> a l b"))
        nc.sync.dma_start(linw_sb[:].rearrange("a (l b) -> a l b", b=128), linw_in.ap().rearrange("l a b -> a l b"))
        nc.sync.dma_start(linbr_sb[:], linbr_in.ap())
        nc.sync.dma_start(linbc_sb[:], linbc_in.ap())
        nc.sync.dma_start(identb[:], identb_in.ap())
        nc.sync.dma_start(identf[:], identf_in.ap())
        nc.sync.dma_start(ones_sb[:], ones_in.ap())
        nc.sync.dma_start(padm_sb[:], padm_in.ap())
        nc.sync.dma_start(xT[:], xT_in.ap())

        # ---- working pools
        PD = ctx.enter_context(tc.tile_pool(name="pd", bufs=2, space="PSUM"))
        PT = ctx.enter_context(tc.tile_pool(name="pt", bufs=2, space="PSUM"))
        PA = ctx.enter_context(tc.tile_pool(name="pa", bufs=2, space="PSUM"))
        PM = ctx.enter_context(tc.tile_pool(name="pm", bufs=2, space="PSUM"))
        GSEG = ctx.enter_context(tc.tile_pool(name="gseg", bufs=2))
        JB = ctx.enter_context(tc.tile_pool(name="jb", bufs=jb_bufs))
        SC = ctx.enter_context(tc.tile_pool(name="sc", bufs=2))
        SS = ctx.enter_context(tc.tile_pool(name="ss", bufs=2))
        RP = ctx.enter_context(tc.tile_pool(name="rp", bufs=1))
        EP = ctx.enter_context(tc.tile_pool(name="ep", bufs=1))

        # idx col offsets (wrapped layout: 8 cols per round)
        idx_off = {}
        ic = 0
        for si, seg in enumerate(segs):
            lo_r = int(sum(D_lo[t] for t in seg))
            hi_r = int(sum(D_hi[t] for t in seg))
            idx_off[si] = (ic, lo_r, ic + 8 * lo_r, hi_r)
            ic += 8 * (lo_r + hi_r)

        for l in range(n_layers):
            wsl = W_sb[:, l * 128:(l + 1) * 128]
            asl = asrc_sb[:, l * 128:(l + 1) * 128]
            adl = adst_sb[:, l * 128:(l + 1) * 128]
            bsl = biasr_sb[:, l * 128:(l + 1) * 128]
            lwl = linw_sb[:, l * 128:(l + 1) * 128]

            # ---- dense: hT = W.T @ xT  (psum chunks of 512)
            for ch0 in range(0, NSHP, 512):
                chsz = min(512, NSHP - ch0)
                pd = PD.tile([128, 512], F32, tag="pd")
                nc.tensor.matmul(pd[:, :chsz], wsl, xT[:, ch0:ch0 + chsz],
                                 start=True, stop=True)
                nc.vector.tensor_copy(hTb[:, ch0:ch0 + chsz], pd[:, :chsz])

            # ---- transpose hT -> h rows (bf16)
            for t in range(TILES):
                sl = slice(t * 128, (t + 1) * 128)
                pt = PT.tile([128, 128], BF16, tag="pt")
                nc.tensor.transpose(pt[:], hTb[:, sl], identb[:])
                nc.vector.tensor_copy(hrows[:, sl], pt[:])

            # ---- AllGather h rows
            if do_collective:
                nc.sync.dma_start(ag_in.ap().rearrange("(t p) f -> p t f", p=128),
                                  hrows[:].rearrange("p (t f) -> p t f", f=128))
                nc.gpsimd.collective_compute(
                    "AllGather", OP.bypass, replica_groups=RG,
                    ins=[ag_in.ap()], outs=[table.ap()])

            # ---- own alpha dots (batched over all tiles) + self weight
            hr3 = hrows[:].rearrange("p (t f) -> p t f", f=128)
            sc3 = hTb[:].rearrange("p (t f) -> p t f", f=128)
            asl3 = asl.rearrange("p (o f) -> p o f", o=1)
            adl3 = adl.rearrange("p (o f) -> p o f", o=1)
            i0, i1 = bass.broadcast_tensor_aps(hr3, asl3)
            nc.vector.tensor_tensor(sc3, i0, i1, op=OP.mult)
            nc.vector.reduce_sum(out=as_own[:], in_=sc3, axis=AX.X)
            i0, i1 = bass.broadcast_tensor_aps(hr3, adl3)
            nc.vector.tensor_tensor(sc3, i0, i1, op=OP.mult)
            nc.vector.reduce_sum(out=ad_own[:], in_=sc3, axis=AX.X)
            zs = SC.tile([128, TILES], F32, tag="zself")
            nc.vector.tensor_tensor(zs[:], as_own[:], ad_own[:], op=OP.add)
            zs2 = SC.tile([128, TILES], F32, tag="zself")
            nc.vector.scalar_tensor_tensor(
                out=zs2[:], in0=zs[:], scalar=NEG, in1=zs[:], op0=OP.mult, op1=OP.max)
            nc.scalar.activation(wself[:], zs2[:], AF.Exp)

            # ---- expand ad_own into gather-column order on the idle ACT
            # engine (per-partition bias broadcast), so the per-segment z2
            # add is one batched DVE op per half instead of 2 ops per tile.
            acol = 0
            for seg_ in segs:
                for D_ in (D_lo, D_hi):
                    for t_ in seg_:
                        d_ = int(D_[t_])
                        nc.scalar.activation(adx[:, acol:acol + d_],
                                             ones_sb[:, :d_], AF.Copy,
                                             scale=ad_own[:, t_:t_ + 1])
                        acol += d_

            # ---- aggregation over segments
            seg_base = 0
            for si, seg in enumerate(segs):
                lo_r = int(sum(D_lo[t] for t in seg))
                hi_r = int(sum(D_hi[t] for t in seg))
                seg_r = lo_r + hi_r
                sb0 = seg_base
                seg_base += seg_r
                gbuf = None
                if do_gather or do_rounds:
                    gbuf = GSEG.tile([128, SEG_MAX_ROUNDS * 128], BF16, tag="gseg")
                    g3 = gbuf[:].rearrange("p (r e) -> p r e", e=128)
                ic_lo, nlo, ic_hi, nhi = idx_off[si]
                if not do_gather and do_rounds:
                    nc.vector.memset(gbuf[:, :seg_r * 128], 0.0)
                if lo_r and do_gather:
                    nc.gpsimd.dma_gather(
                        g3[:, :lo_r, :], table.ap()[:32768, :],
                        idx_sb[:, ic_lo:ic_lo + 8 * lo_r],
                        128 * lo_r, 128 * lo_r, 128, single_packet=single_packet)
                if hi_r and do_gather:
                    nc.gpsimd.dma_gather(
                        g3[:, lo_r:seg_r, :], table.ap()[HI_BASE:TBL, :],
                        idx_sb[:, ic_hi:ic_hi + 8 * hi_r],
                        128 * hi_r, 128 * hi_r, 128, single_packet=single_packet)

                if not do_rounds:
                    for t in seg:
                        tsl = slice(t * 128, (t + 1) * 128)
                        row = RP.tile([128, 128], F32, tag="row")
                        nc.vector.tensor_copy(row[:], hrows[:, tsl])
                        ptf = PM.tile([128, 128], F32, tag="pm")
                        nc.tensor.transpose(ptf[:], row[:], identf[:])
                        nc.vector.tensor_copy(xT[:, tsl], ptf[:])
                    continue

                # ---- batched attention logits, split lo/hi so the lo-half
                # chain overlaps the hi gather (subtile deps give the early
                # start: the lo ops read only the lo region of gbuf).
                jb = JB.tile([128, SEG_MAX_ROUNDS * 128], BF16, tag="jb")
                z = SC.tile([128, SEG_MAX_ROUNDS], F32, tag="z")
                z2 = SC.tile([128, SEG_MAX_ROUNDS], F32, tag="z2")
                zl = SC.tile([128, SEG_MAX_ROUNDS], F32, tag="zl")
                ew = SC.tile([128, SEG_MAX_ROUNDS], F32, tag="ew")
                ewm = SC.tile([128, SEG_MAX_ROUNDS], F32, tag="ewm")
                for r0, r1 in ((0, lo_r), (lo_r, seg_r)):
                    if r0 == r1:
                        continue
                    gp = gbuf[:, r0 * 128:r1 * 128].rearrange(
                        "p (r e) -> p r e", e=128)
                    jp = jb[:, r0 * 128:r1 * 128].rearrange(
                        "p (r e) -> p r e", e=128)
                    i0, i1 = bass.broadcast_tensor_aps(gp, asl3)
                    nc.vector.tensor_tensor(jp, i0, i1, op=OP.mult)
                    nc.vector.reduce_sum(out=z[:, r0:r1], in_=jp, axis=AX.X)
                    nc.vector.tensor_tensor(
                        z2[:, r0:r1], z[:, r0:r1],
                        adx[:, sb0 + r0:sb0 + r1], op=OP.add)
                    nc.vector.scalar_tensor_tensor(
                        out=zl[:, r0:r1], in0=z2[:, r0:r1], scalar=NEG,
                        in1=z2[:, r0:r1], op0=OP.mult, op1=OP.max)
                    nc.scalar.activation(ew[:, r0:r1], zl[:, r0:r1], AF.Exp)
                    nc.vector.tensor_tensor(
                        ewm[:, r0:r1], ew[:, r0:r1],
                        mask_sb[:, sb0 + r0:sb0 + r1], op=OP.mult)
                    # weight apply for this half: jb <- gbuf * ewm (broadcast)
                    ep = ewm[:, r0:r1].rearrange("p (r o) -> p r o", o=1)
                    i0, i1 = bass.broadcast_tensor_aps(gp, ep)
                    nc.vector.tensor_tensor(jp, i0, i1, op=OP.mult)

                # ---- per-tile accumulate; epilogue math batched per segment
                n_t = len(seg)
                t0 = seg[0]
                s1g = SC.tile([128, MAXNT], F32, tag="s1g")
                s2g = SC.tile([128, MAXNT], F32, tag="s2g")
                paseg = EP.tile([128, MAXNT * 128], F32, tag="paseg")
                lo_c = 0
                hi_c = 0
                for ti, t in enumerate(seg):
                    dlo, dhi = int(D_lo[t]), int(D_hi[t])
                    Dt = dlo + dhi
                    nc.vector.reduce_sum(out=s1g[:, ti:ti + 1],
                                         in_=ewm[:, lo_c:lo_c + dlo], axis=AX.X)
                    nc.vector.reduce_sum(
                        out=s2g[:, ti:ti + 1],
                        in_=ewm[:, lo_r + hi_c:lo_r + hi_c + dhi], axis=AX.X)
                    pa = PA.tile([128, 128], F32, tag="pa")
                    for k in range(Dt):
                        c = (lo_c + k) if k < dlo else (lo_r + hi_c + k - dlo)
                        nc.tensor.matmul(pa[:], identb[:],
                                         jb[:, c * 128:(c + 1) * 128],
                                         start=(k == 0), stop=(k == Dt - 1))
                    nc.vector.tensor_copy(paseg[:, ti * 128:(ti + 1) * 128], pa[:])
                    lo_c += dlo
                    hi_c += dhi

                # softmax denominators for all tiles of the segment at once
                spg = SC.tile([128, MAXNT], F32, tag="spg")
                nc.vector.scalar_tensor_tensor(
                    out=spg[:, :n_t], in0=s1g[:, :n_t], scalar=SEPS,
                    in1=s2g[:, :n_t], op0=OP.add, op1=OP.add)
                sp2g = SC.tile([128, MAXNT], F32, tag="sp2g")
                nc.vector.tensor_tensor(sp2g[:, :n_t], spg[:, :n_t],
                                        wself[:, t0:t0 + n_t], op=OP.add)
                recg = SC.tile([128, MAXNT], F32, tag="recg")
                nc.vector.reciprocal(recg[:, :n_t], sp2g[:, :n_t])

                # acc = hrows*wself + pa ; row = acc*rec + bias  (3D bcast)
                ssl = slice(t0 * 128, (t0 + n_t) * 128)
                hr3s = hrows[:, ssl].rearrange("p (t f) -> p t f", f=128)
                ws3 = wself[:, t0:t0 + n_t].rearrange("p (t o) -> p t o", o=1)
                acc1g = EP.tile([128, MAXNT * 128], F32, tag="acc1g")
                a3 = acc1g[:, :n_t * 128].rearrange("p (t f) -> p t f", f=128)
                i0, i1 = bass.broadcast_tensor_aps(hr3s, ws3)
                nc.vector.tensor_tensor(a3, i0, i1, op=OP.mult)
                rowg = EP.tile([128, MAXNT * 128], F32, tag="rowg")
                nc.vector.tensor_tensor(rowg[:, :n_t * 128], acc1g[:, :n_t * 128],
                                        paseg[:, :n_t * 128], op=OP.add)
                r3 = rowg[:, :n_t * 128].rearrange("p (t f) -> p t f", f=128)
                rec3 = recg[:, :n_t].rearrange("p (t o) -> p t o", o=1)
                i0, i1 = bass.broadcast_tensor_aps(r3, rec3)
                nc.vector.tensor_tensor(a3, i0, i1, op=OP.mult)
                b3 = bsl.rearrange("p (o f) -> p o f", o=1)
                i0, i1 = bass.broadcast_tensor_aps(a3, b3)
                nc.vector.tensor_tensor(r3, i0, i1, op=OP.add)
                if seg[-1] == TILES - 1:
                    lsl = slice((n_t - 1) * 128, n_t * 128)
                    nc.vector.tensor_scalar_mul(rowg[:, lsl], rowg[:, lsl],
                                                padm_sb[:])
                # transpose pre-pairnorm rows into xT slab
                for ti, t in enumerate(seg):
                    ptf = PM.tile([128, 128], F32, tag="pm")
                    nc.tensor.transpose(ptf[:],
                                        rowg[:, ti * 128:(ti + 1) * 128],
                                        identf[:])
                    nc.vector.tensor_copy(xT[:, t * 128:(t + 1) * 128], ptf[:])

            # ---- pairnorm stats (on transposed pre-pairnorm slab)
            fsum = SS.tile([128, 1], F32, tag="fsum")
            nc.vector.reduce_sum(out=fsum[:], in_=xT[:], axis=AX.X)
            sqc = SS.tile([128, 1], F32, tag="sqc")
            nc.scalar.activation(hTb[:], xT[:], AF.Square, accum_out=sqc[:])
            nc.vector.tensor_copy(stp[:, :1], fsum[:])
            nc.vector.tensor_copy(stp[:, 1:2], sqc[:])
            if do_stats:
                nc.sync.dma_start(st_in.ap(), stp[:])
                nc.gpsimd.collective_compute(
                    "AllGather", OP.bypass, replica_groups=RG,
                    ins=[st_in.ap()], outs=[st_out.ap()])
                nc.sync.dma_start(gtmp[:].rearrange("p (r c) -> p r c", c=2),
                                  st_out.ap().rearrange("(r p) c -> p r c", p=128))
                nc.vector.reduce_sum(out=gstat[:],
                                     in_=gtmp[:].rearrange("p (r c) -> p c r", c=2),
                                     axis=AX.X)
            else:
                nc.vector.tensor_scalar_mul(gstat[:], stp[:], float(NC))
            mu = SS.tile([128, 1], F32, tag="mu")
            nc.vector.tensor_scalar_mul(mu[:], gstat[:, :1], 1.0 / N)
            st2 = SS.tile([128, 2], F32, tag="st2")
            nc.vector.tensor_copy(st2[:, :1], gstat[:, 1:2])
            nc.vector.tensor_tensor(st2[:, 1:2], mu[:], mu[:], op=OP.mult)
            p2 = PM.tile([128, 128], F32, tag="pm")
            nc.tensor.matmul(p2[:1, :2], ones_sb[:, :1], st2[:],
                             start=True, stop=True)
            tot = SS.tile([1, 2], F32, tag="tot")
            nc.vector.tensor_copy(tot[:], p2[:1, :2])
            v3 = SS.tile([1, 1], F32, tag="v3")
            nc.vector.tensor_scalar(v3[:], tot[:, :1], 1.0 / N, PEPS,
                                    op0=OP.mult, op1=OP.add)
            v4 = SS.tile([1, 1], F32, tag="v4")
            nc.vector.tensor_tensor(v4[:], v3[:], tot[:, 1:2], op=OP.subtract)
            den = SS.tile([1, 1], F32, tag="den")
            nc.scalar.activation(den[:], v4[:], AF.Sqrt)
            invd = SS.tile([1, 1], F32, tag="invd")
            nc.vector.reciprocal(invd[:], den[:])
            pb1 = PM.tile([128, 128], F32, tag="pm")
            nc.tensor.matmul(pb1[:, :1], ones_sb[:1, :], invd[:],
                             start=True, stop=True)
            invdr = SS.tile([128, 1], F32, tag="invdr")
            nc.vector.tensor_copy(invdr[:], pb1[:, :1])
            nms = SS.tile([128, 1], F32, tag="nms")
            nc.vector.tensor_scalar(nms[:], mu[:], invdr[:], -1.0,
                                    op0=OP.mult, op1=OP.mult)

            # ---- pairnorm + gelu fused on ACT (in-place on xT)
            for ch0 in range(0, NSHP, 512):
                chsz = min(512, NSHP - ch0)
                csl = slice(ch0, ch0 + chsz)
                nc.scalar.activation(xT[:, csl], xT[:, csl], GELU,
                                     bias=nms[:], scale=invdr[:])

            # ---- final-linear increment (outfin holds y^T; fixed stationary)
            for ch0 in range(0, NSHP, 512):
                chsz = min(512, NSHP - ch0)
                csl = slice(ch0, ch0 + chsz)
                pf = PD.tile([128, 512], F32, tag="pd")
                nc.tensor.matmul(pf[:, :chsz], lwl, xT[:, csl],
                                 start=True, stop=True)
                if l == 0:
                    nc.vector.tensor_scalar_add(outfin[:, csl], pf[:, :chsz],
                                                linbc_sb[:])
                else:
                    nc.vector.scalar_tensor_tensor(
                        out=outfin[:, csl], in0=pf[:, :chsz], scalar=1.0,
                        in1=outfin[:, csl], op0=OP.mult, op1=OP.add)

        # ---- write final output (transpose y^T back to node rows)
        for t0 in range(0, TILES, MAXNT):
            ng = min(MAXNT, TILES - t0)
            rT = EP.tile([128, MAXNT * 128], F32, tag="rowg")
            for q in range(ng):
                qsl = slice((t0 + q) * 128, (t0 + q + 1) * 128)
                ptf = PM.tile([128, 128], F32, tag="pm")
                nc.tensor.transpose(ptf[:], outfin[:, qsl], identf[:])
                nc.vector.tensor_copy(rT[:, q * 128:(q + 1) * 128], ptf[:])
            nc.sync.dma_start(
                y_out.ap()[t0 * 128:(t0 + ng) * 128, :].rearrange(
                    "(t p) f -> p t f", p=128),
                rT[:, :ng * 128].rearrange("p (t f) -> p t f", f=128))

    return nc


def make_inputs(inputs, meta, percore, n_layers=L):
    """Build per-core in_maps from the full problem inputs."""
    x = np.asarray(inputs["x"], np.float32)
    W0 = np.asarray(inputs["W0"], np.float32)
    Ws = np.asarray(inputs["Ws"], np.float32)
    att_src = np.asarray(inputs["att_src"], np.float32)
    att_dst = np.asarray(inputs["att_dst"], np.float32)
    bias = np.asarray(inputs["bias"], np.float32)
    lin_w = np.asarray(inputs["lin_w"], np.float32)
    lin_b = np.asarray(inputs["lin_b"], np.float32)

    Wst = np.stack([W0] + [Ws[i] for i in range(n_layers - 1)]).astype(np.float32)
    asrc = np.stack([np.tile(att_src[i], (128, 1)) for i in range(n_layers)]).astype(ml_dtypes.bfloat16)
    adst = np.stack([np.tile(att_dst[i], (128, 1)) for i in range(n_layers)]).astype(ml_dtypes.bfloat16)
    biasr = np.stack([np.tile(bias[i], (128, 1)) for i in range(n_layers)]).astype(np.float32)
    linw = np.stack([lin_w[i * HID:(i + 1) * HID] for i in range(n_layers)]).astype(np.float32)
    linbr = np.tile(lin_b, (128, 1)).astype(np.float32)
    identb = np.eye(128, dtype=ml_dtypes.bfloat16)
    identf = np.eye(128, dtype=np.float32)
    ones = np.ones((128, 128), np.float32)
    padm = np.zeros((128, 1), np.float32)
    padm[:NSH - (TILES - 1) * 128] = 1.0

    in_maps = []
    for c in range(NC):
        order = percore[c]["order"]
        xs = x[c * NSH:(c + 1) * NSH][order]                       # [6250,128] sorted
        xT = np.zeros((128, NSHP), np.float32)
        xT[:, :NSH] = xs.T
        in_maps.append({
            "xT": xT, "idx": percore[c]["idx"], "mask": percore[c]["mask"].astype(ml_dtypes.bfloat16),
            "Wst": Wst, "asrc": asrc, "adst": adst, "biasr": biasr,
            "linw": linw, "linbr": linbr,
            "linbc": lin_b.reshape(128, 1).astype(np.float32),
            "identb": identb, "identf": identf,
            "ones": ones, "padm": padm,
        })
    return in_maps


def assemble_output(results, percore):
    """Concatenate per-core outputs, undoing the degree-sort permutation."""
    out = np.empty((N, HID), np.float32)
    for c in range(NC):
        order = percore[c]["order"]
        yc = results[c]["y"][:NSH]     # sorted order
        out[c * NSH + order] = yc
    return out


# ---------------------------------------------------------------------------
# kernel() entry point
# ---------------------------------------------------------------------------
_CACHE = {}


def _get_compiled(edge_key, edge_index):
    if edge_key not in _CACHE:
        meta, percore = preprocess(edge_index)
        nc = bacc.Bacc("TRN2", target_bir_lowering=False, debug=False,
                       num_devices=NC)
        build(nc, meta, n_layers=L, sim_safe=False)
        nc.compile()
        _CACHE[edge_key] = (nc, meta, percore)
    return _CACHE[edge_key]


def kernel(**inputs):
    from concourse.bass_utils import run_bass_kernel_spmd
    edge_index = np.asarray(inputs["edge_index"])
    edge_key = hash(edge_index.tobytes())
    nc, meta, percore = _get_compiled(edge_key, edge_index)
    in_maps = make_inputs(inputs, meta, percore, n_layers=L)
    res = run_bass_kernel_spmd(nc, in_maps, list(range(NC)))
    return assemble_output(res.results, percore)

